# revision 1
# baseline (speedup 1.0000x reference)
import sys
sys.path.insert(0, "/opt/trn_rl_repo")
import time
import numpy as np
import ml_dtypes

N_NODES = 131072
N_EDGES = 2097152
N_GRAPHS = 2048
IN_CH, HID, OUT = 12, 64, 4
NCORES = 8
COLS = 132            # 132*128 = 16896 node capacity per shard
NL = COLS * 128
PAD_ROW = N_NODES     # zero row in tables

_prog_cache = {}

LAST_EXEC_WALLS = []


def _build_launch(D, slot_cols):
    import concourse.bass as bass
    import concourse.bacc as bacc
    import concourse.tile as tile
    import concourse.mybir as mybir
    n_slots = len(slot_cols)
    nc = bacc.Bacc("TRN2", target_bir_lowering=False, debug=False, num_devices=NCORES)
    tab = nc.dram_tensor("tab", [N_NODES + 1, D], mybir.dt.bfloat16, kind="ExternalInput").ap()
    idx = nc.dram_tensor("idx", [128, n_slots], mybir.dt.int32, kind="ExternalInput").ap()
    selfv = nc.dram_tensor("selfv", [128, COLS * D], mybir.dt.float32, kind="ExternalInput").ap()
    dinv = nc.dram_tensor("dinv", [128, COLS], mybir.dt.float32, kind="ExternalInput").ap()
    outp = nc.dram_tensor("outp", [NL, D], mybir.dt.float32, kind="ExternalOutput").ap()
    with tile.TileContext(nc) as tc:
        with tc.tile_pool(name="p", bufs=1) as pool, tc.tile_pool(name="o", bufs=4) as opool:
            idx_t = pool.tile([128, n_slots], mybir.dt.int32, name="idxt")
            nc.sync.dma_start(idx_t[:], idx[:])
            self_t = pool.tile([128, COLS * D], mybir.dt.float32, name="selft")
            nc.sync.dma_start(self_t[:], selfv[:])
            dinv_t = pool.tile([128, COLS], mybir.dt.float32, name="dinvt")
            nc.sync.dma_start(dinv_t[:], dinv[:])
            A = [pool.tile([128, D], mybir.dt.float32, name=f"A{c}", tag=f"A{c}")
                 for c in range(COLS)]
            for c in range(COLS):
                nc.vector.memset(A[c][:], 0.0)
            # gather-and-accumulate: one instruction adds table rows for 128 nodes
            for pos, c in enumerate(slot_cols):
                nc.gpsimd.indirect_dma_start(
                    out=A[c][:, :],
                    out_offset=None,
                    in_=tab[:],
                    in_offset=bass.IndirectOffsetOnAxis(ap=idx_t[:, pos:pos + 1], axis=0),
                    compute_op=mybir.AluOpType.add,
                )
            for c in range(COLS):
                nc.vector.tensor_add(A[c][:], A[c][:], self_t[:, c * D:(c + 1) * D])
                ot = opool.tile([128, D], mybir.dt.float32, name="ot", tag="ot")
                nc.scalar.mul(ot[:], A[c][:], dinv_t[:, c:c + 1])
                nc.sync.dma_start(outp[c * 128:(c + 1) * 128, :], ot[:])
    nc.compile()
    return nc


def _run_launch(nc, in_maps):
    from concourse.bass_utils import run_bass_kernel_spmd
    t0 = time.perf_counter()
    res = run_bass_kernel_spmd(nc, in_maps, core_ids=list(range(NCORES)))
    LAST_EXEC_WALLS.append(time.perf_counter() - t0)
    return [r["outp"] for r in res.results]


def kernel(x, edge_index, batch, W1, b1, W2, b2, Wfc, bfc):
    x = np.asarray(x, np.float32)
    src = np.asarray(edge_index[0], np.int64).astype(np.int32)
    dst = np.asarray(edge_index[1], np.int64).astype(np.int32)
    batch = np.asarray(batch, np.int64).astype(np.int32)
    W1 = np.asarray(W1, np.float32); b1 = np.asarray(b1, np.float32)
    W2 = np.asarray(W2, np.float32); b2 = np.asarray(b2, np.float32)
    Wfc = np.asarray(Wfc, np.float32); bfc = np.asarray(bfc, np.float32)

    # ---------- host index preprocessing ----------
    deg = np.bincount(dst, minlength=N_NODES).astype(np.float32) + 1.0
    dinv = 1.0 / np.sqrt(deg)

    order = np.argsort(dst, kind="stable")
    dst_s = dst[order]; src_s = src[order]
    starts = np.searchsorted(dst_s, np.arange(N_NODES + 1)).astype(np.int64)

    # graph-aligned shard boundaries near multiples of N_NODES/8
    gcnt = np.bincount(batch, minlength=N_GRAPHS)
    gcum = np.concatenate([[0], np.cumsum(gcnt)])  # node index at graph starts
    bounds = [0]
    for d in range(1, NCORES):
        tgt = d * (N_NODES // NCORES)
        g = np.argmin(np.abs(gcum - tgt))
        bounds.append(int(gcum[g]))
    bounds.append(N_NODES)

    shards = []
    colmax = np.zeros((NCORES, COLS), np.int64)
    for d in range(NCORES):
        s_d, e_d = bounds[d], bounds[d + 1]
        nloc = e_d - s_d
        assert nloc <= NL, (nloc, NL)
        ldeg = (starts[s_d + 1:e_d + 1] - starts[s_d:e_d]).astype(np.int64)
        rank_to_local = np.argsort(-ldeg, kind="stable")
        rdeg = ldeg[rank_to_local]
        rdeg_pad = np.zeros(NL, np.int64)
        rdeg_pad[:nloc] = rdeg
        colmax[d] = rdeg_pad.reshape(COLS, 128).max(axis=1)
        shards.append((s_d, e_d, nloc, rank_to_local, ldeg))
    K_c = colmax.max(axis=0)          # shared slot structure
    slots = []                        # j-major emission order
    for j in range(int(K_c.max())):
        for c in range(COLS):
            if K_c[c] > j:
                slots.append((c, j))
    slot_cols = [c for c, _ in slots]
    n_slots = len(slots)

    idx_arrs = []
    rank_gn = []
    for d in range(NCORES):
        s_d, e_d, nloc, rank_to_local, ldeg = shards[d]
        gn_of_rank = np.full(NL, -1, np.int64)
        gn_of_rank[:nloc] = s_d + rank_to_local
        rank_gn.append(gn_of_rank)
        ia = np.full((128, n_slots), PAD_ROW, np.int32)
        p_idx = np.arange(128)
        for pos, (c, j) in enumerate(slots):
            gn = gn_of_rank[c * 128 + p_idx]
            ok = gn >= 0
            gok = gn[ok]
            dok = (starts[gok + 1] - starts[gok]) > j
            sel = np.where(ok)[0][dok]
            ia[sel, pos] = src_s[starts[gn[sel]] + j].astype(np.int32)
        idx_arrs.append(ia)

    def pack_rank_rows(vals_global, d, D):
        # vals_global: [N_NODES, D] -> [128, COLS*D] in rank layout (node rank r -> partition r%128, col r//128)
        gn = rank_gn[d]
        out = np.zeros((NL, D), np.float32)
        ok = gn >= 0
        out[ok] = vals_global[gn[ok]]
        return out.reshape(COLS, 128, D).transpose(1, 0, 2).reshape(128, COLS * D)

    def unpack_rank_rows(flat_rows, d, D):
        # [NL, D] device output (row r = rank r? rows are c*128+p) -> global [N_NODES slice]
        gn = rank_gn[d]
        vals = np.zeros((N_NODES, D), np.float32)
        ok = gn >= 0
        vals[gn[ok]] = flat_rows[ok]
        return vals

    dinv_rank = []
    for d in range(NCORES):
        gn = rank_gn[d]
        dv = np.zeros(NL, np.float32)
        ok = gn >= 0
        dv[ok] = dinv[gn[ok]]
        dinv_rank.append(dv.reshape(COLS, 128).T.copy())   # [128, COLS]

    # ---------- launch 1: aggregate x' = dinv*x ----------
    key1 = ("L1", IN_CH, tuple(slot_cols))
    if key1 not in _prog_cache:
        _prog_cache[key1] = _build_launch(IN_CH, slot_cols)
    nc1 = _prog_cache[key1]

    xp = x * dinv[:, None]                       # x' (fp32)
    tab1 = np.zeros((N_NODES + 1, IN_CH), ml_dtypes.bfloat16)
    tab1[:N_NODES] = xp.astype(ml_dtypes.bfloat16)
    in_maps1 = []
    for d in range(NCORES):
        in_maps1.append({
            "tab": tab1,
            "idx": idx_arrs[d],
            "selfv": pack_rank_rows(xp, d, IN_CH),
            "dinv": dinv_rank[d],
        })
    outs1 = _run_launch(nc1, in_maps1)           # [NL, IN_CH] = dinv*(A1 + x') rank rows

    # ---------- host: tiny dense matmuls between layers ----------
    P2 = np.zeros((N_NODES, HID), np.float32)
    for d in range(NCORES):
        a = outs1[d]                              # [NL, IN_CH] rank rows
        t2 = np.maximum(a @ W1 + b1, 0.0)         # relu(out1_pre @ W1 + b1)
        p2r = t2 @ W2
        gn = rank_gn[d]
        ok = gn >= 0
        P2[gn[ok]] = p2r[ok] * dinv[gn[ok]][:, None]
    tab2 = np.zeros((N_NODES + 1, HID), ml_dtypes.bfloat16)
    tab2[:N_NODES] = P2.astype(ml_dtypes.bfloat16)

    # ---------- launch 2: aggregate P2 ----------
    key2 = ("L2", HID, tuple(slot_cols))
    if key2 not in _prog_cache:
        _prog_cache[key2] = _build_launch(HID, slot_cols)
    nc2 = _prog_cache[key2]

    in_maps2 = []
    for d in range(NCORES):
        in_maps2.append({
            "tab": tab2,
            "idx": idx_arrs[d],
            "selfv": pack_rank_rows(P2, d, HID),
            "dinv": dinv_rank[d],
        })
    outs2 = _run_launch(nc2, in_maps2)           # [NL, HID] = dinv*(A2 + P2) rank rows

    # ---------- host: bias+relu, pooling, FC, sigmoid ----------
    out2 = np.zeros((N_NODES, HID), np.float32)
    for d in range(NCORES):
        out2 += unpack_rank_rows(np.maximum(outs2[d] + b2, 0.0) * (rank_gn[d][:, None] >= 0), d, HID)
    sums = np.zeros((N_GRAPHS, HID), np.float32)
    np.add.at(sums, batch, out2)
    cnt = np.bincount(batch, minlength=N_GRAPHS).astype(np.float32)
    g = sums / np.maximum(cnt, 1.0)[:, None]
    logits = g @ Wfc + bfc
    return (1.0 / (1.0 + np.exp(-logits))).astype(np.float32)



# revision 11
# speedup vs baseline: 8.3809x; 8.3809x over previous
import sys
sys.path.insert(0, "/opt/trn_rl_repo")
import time
import numpy as np
import ml_dtypes

# ---- problem constants (hardcoded; kernel.py must be self-contained) ----
N_NODES = 131072
N_EDGES = 2097152
N_GRAPHS = 2048
IN_CH, HID, OUT = 12, 64, 4
NCORES = 8
NLOC = N_NODES // NCORES      # 16384 nodes per core (fixed ranges)
COLS = NLOC // 128            # 128 columns of 128 nodes
PAD_ROW = N_NODES             # zero row appended to the gather tables

# Static per-column slot capacities (max in-degree of the column after
# degree-sorting node ranks, maxed over the 8 shards) for the reference
# input distribution, plus safety margin. If actual inputs exceed this,
# a custom program is built at runtime (slow path, still correct).
_K_SEED = [36, 27, 25, 25, 24, 24, 23, 23, 22, 22, 22, 22, 21, 21, 21, 21,
           21, 21, 20, 20, 20, 20, 20, 20, 20, 19, 19, 19, 19, 19, 19, 19,
           19, 19, 18, 18, 18, 18, 18, 18, 18, 18, 18, 18, 18, 17, 17, 17,
           17, 17, 17, 17, 17, 17, 17, 17, 16, 16, 16, 16, 16, 16, 16, 16,
           16, 16, 16, 16, 16, 15, 15, 15, 15, 15, 15, 15, 15, 15, 15, 15,
           15, 15, 15, 14, 14, 14, 14, 14, 14, 14, 14, 14, 14, 14, 13, 13,
           13, 13, 13, 13, 13, 13, 13, 13, 12, 12, 12, 12, 12, 12, 12, 12,
           12, 11, 11, 11, 11, 11, 11, 10, 10, 10, 10, 9, 9, 9, 8, 7]
K_STATIC = tuple(k + 3 for k in _K_SEED)
GSLOT_STATIC = 384            # graph slots per core (3 PSUM tiles of 128)

LAST_EXEC_WALLS = []
_prog_cache = {}


def _slot_schedule(K):
    """j-major slot emission order + per-(col, j) position table."""
    slots = []
    slotpos = np.full((COLS, max(K)), -1, np.int64)
    for j in range(max(K)):
        for c in range(COLS):
            if K[c] > j:
                slotpos[c, j] = len(slots)
                slots.append((c, j))
    return slots, slotpos


def _build_program(K, gslot, _stage=99, _compile=True):
    import concourse.bass as bass
    import concourse.bacc as bacc
    import concourse.tile as tile
    import concourse.mybir as mybir
    from concourse.masks import make_identity

    slots, _ = _slot_schedule(K)
    nslots = len(slots)
    gtiles = gslot // 128
    assert gslot % 128 == 0

    nc = bacc.Bacc("TRN2", target_bir_lowering=False, debug=False,
                   num_devices=NCORES)
    f32, bf16, i32 = mybir.dt.float32, mybir.dt.bfloat16, mybir.dt.int32

    xpT = nc.dram_tensor("xpT", [IN_CH, NLOC], bf16, kind="ExternalInput").ap()
    idx = nc.dram_tensor("idx", [128, nslots], i32, kind="ExternalInput").ap()
    dinv = nc.dram_tensor("dinv", [128, COLS], f32, kind="ExternalInput").ap()
    bslot = nc.dram_tensor("bslot", [128, COLS], f32, kind="ExternalInput").ap()
    w1 = nc.dram_tensor("w1", [IN_CH, HID], bf16, kind="ExternalInput").ap()
    w2 = nc.dram_tensor("w2", [HID, HID], bf16, kind="ExternalInput").ap()
    b1 = nc.dram_tensor("b1", [HID, 1], f32, kind="ExternalInput").ap()
    b2rep = nc.dram_tensor("b2rep", [128, HID], f32, kind="ExternalInput").ap()

    t1loc = nc.dram_tensor("t1loc", [NLOC, HID], bf16, kind="Internal").ap()
    t2loc = nc.dram_tensor("t2loc", [NLOC, HID], bf16, kind="Internal").ap()
    t1full = nc.dram_tensor("t1full", [N_NODES + 1, HID], bf16,
                            kind="Internal", addr_space="Shared").ap()
    t2full = nc.dram_tensor("t2full", [N_NODES + 1, HID], bf16,
                            kind="Internal", addr_space="Shared").ap()
    poolsum = nc.dram_tensor("poolsum", [gslot, HID], f32,
                             kind="ExternalOutput").ap()

    RG = [list(range(NCORES))]
    relu = mybir.ActivationFunctionType.Relu

    with tile.TileContext(nc) as tc:
        with tc.tile_pool(name="const", bufs=1) as cpool, \
             tc.tile_pool(name="work", bufs=3) as wpool:
            # ---- constant loads ----
            idx_t = cpool.tile([128, nslots], i32, name="idx_t")
            nc.sync.dma_start(idx_t[:], idx[:])
            xpT_t = cpool.tile([IN_CH, NLOC], bf16, name="xpT_t")
            nc.sync.dma_start(xpT_t[:], xpT[:])
            dinv_t = cpool.tile([128, COLS], f32, name="dinv_t")
            nc.sync.dma_start(dinv_t[:], dinv[:])
            bslot_t = cpool.tile([128, COLS], f32, name="bslot_t")
            nc.sync.dma_start(bslot_t[:], bslot[:])
            w1_t = cpool.tile([IN_CH, HID], bf16, name="w1_t")
            nc.sync.dma_start(w1_t[:], w1[:])
            w2_t = cpool.tile([HID, HID], bf16, name="w2_t")
            nc.sync.dma_start(w2_t[:], w2[:])
            b1_t = cpool.tile([HID, 1], f32, name="b1_t")
            nc.sync.dma_start(b1_t[:], b1[:])
            b2rep_t = cpool.tile([128, HID], f32, name="b2rep_t")
            nc.sync.dma_start(b2rep_t[:], b2rep[:])

            ident = cpool.tile([128, 128], f32, name="ident")
            make_identity(nc, ident[:])
            iota_i = cpool.tile([128, gslot], i32, name="iota_i")
            nc.gpsimd.iota(iota_i[:], pattern=[[1, gslot]], base=0,
                           channel_multiplier=0)
            iota_f = cpool.tile([128, gslot], f32, name="iota_f")
            nc.vector.tensor_copy(iota_f[:], iota_i[:])

            zrow = cpool.tile([1, HID], bf16, name="zrow")
            nc.vector.memset(zrow[:], 0.0)
            nc.sync.dma_start(t1full[PAD_ROW:PAD_ROW + 1, :], zrow[:])
            nc.sync.dma_start(t2full[PAD_ROW:PAD_ROW + 1, :], zrow[:])

            ylocal = cpool.tile([128, COLS * HID], bf16, name="ylocal")
            t2local = cpool.tile([128, COLS * HID], bf16, name="t2local")
            A = [cpool.tile([128, HID], f32, name=f"A{c}") for c in range(COLS)]
            B = [cpool.tile([128, HID], f32, name=f"B{c}") for c in range(COLS)]
            for c in range(COLS):
                nc.vector.memset(A[c][:], 0.0)
                nc.vector.memset(B[c][:], 0.0)

            # ---- stage A: y = xp @ W1 per column (all in transposed form) ----
            with tc.tile_pool(name="psA", bufs=4, space="PSUM") as psA:
                if _stage < 2:
                    slots2, cols2 = [], 0
                else:
                    slots2, cols2 = slots, COLS
                for c in range(cols2):
                    yp = psA.tile([128, HID], f32, space="PSUM", tag="yp")
                    nc.tensor.matmul(out=yp[:], lhsT=xpT_t[:, c * 128:(c + 1) * 128],
                                     rhs=w1_t[:], start=True, stop=True)
                    nc.scalar.activation(ylocal[:, c * HID:(c + 1) * HID], yp[:],
                                         mybir.ActivationFunctionType.Copy)
            # write T1 local chunk (row = p*COLS + c) and all-gather
            t1loc_v = t1loc[:, :].rearrange("(p c) f -> p (c f)", p=128)
            if _stage >= 2:
                nc.sync.dma_start(t1loc_v, ylocal[:])
            if _stage >= 3:
                nc.gpsimd.collective_compute(
                    kind="AllGather", op=mybir.AluOpType.bypass, replica_groups=RG,
                    ins=[t1loc[:, :]], outs=[t1full[0:N_NODES, :]])

            # ---- stage C: layer-1 gather-accumulate ----
            for pos, (c, j) in (enumerate(slots) if _stage >= 4 else []):
                nc.gpsimd.indirect_dma_start(
                    out=A[c][:, :], out_offset=None, in_=t1full[:],
                    in_offset=bass.IndirectOffsetOnAxis(
                        ap=idx_t[:, pos:pos + 1], axis=0),
                    compute_op=mybir.AluOpType.add)

            with tc.tile_pool(name="psC", bufs=4, space="PSUM") as psC:
                for c in (range(COLS) if _stage >= 5 else []):
                    cs = slice(c * HID, (c + 1) * HID)
                    nc.vector.tensor_add(A[c][:], A[c][:], ylocal[:, cs])
                    nc.scalar.mul(A[c][:], A[c][:], dinv_t[:, c:c + 1])
                    tp = psC.tile([HID, 128], f32, space="PSUM", tag="tp")
                    nc.tensor.transpose(tp[:], A[c][:], ident[:])
                    h1T = wpool.tile([HID, 128], bf16, tag="h1T")
                    nc.scalar.activation(h1T[:], tp[:], relu, bias=b1_t[:, 0:1])
                    t2p = psC.tile([128, HID], f32, space="PSUM", tag="t2p")
                    nc.tensor.matmul(out=t2p[:], lhsT=h1T[:], rhs=w2_t[:],
                                     start=True, stop=True)
                    nc.scalar.mul(t2local[:, cs], t2p[:], dinv_t[:, c:c + 1])

            t2loc_v = t2loc[:, :].rearrange("(p c) f -> p (c f)", p=128)
            if _stage >= 6:
                nc.sync.dma_start(t2loc_v, t2local[:])
                nc.gpsimd.collective_compute(
                    kind="AllGather", op=mybir.AluOpType.bypass, replica_groups=RG,
                    ins=[t2loc[:, :]], outs=[t2full[0:N_NODES, :]])

            # ---- stage E: layer-2 gather-accumulate ----
            for pos, (c, j) in (enumerate(slots) if _stage >= 7 else []):
                nc.gpsimd.indirect_dma_start(
                    out=B[c][:, :], out_offset=None, in_=t2full[:],
                    in_offset=bass.IndirectOffsetOnAxis(
                        ap=idx_t[:, pos:pos + 1], axis=0),
                    compute_op=mybir.AluOpType.add)

            with tc.tile_pool(name="psE", bufs=1, space="PSUM") as psE:
                poolp = [psE.tile([128, HID], f32, space="PSUM", name=f"pool{t}")
                         for t in range(gtiles)]
                for c in (range(COLS) if _stage >= 8 else []):
                    cs = slice(c * HID, (c + 1) * HID)
                    nc.vector.tensor_add(B[c][:], B[c][:], t2local[:, cs])
                    nc.scalar.mul(B[c][:], B[c][:], dinv_t[:, c:c + 1])
                    h2 = wpool.tile([128, HID], f32, tag="h2")
                    nc.vector.tensor_add(h2[:], B[c][:], b2rep_t[:])
                    h2b = wpool.tile([128, HID], bf16, tag="h2b")
                    nc.vector.tensor_scalar_max(h2b[:], h2[:], 0.0)
                    for t in range(gtiles):
                        oh = wpool.tile([128, 128], bf16, tag=f"oh{t}")
                        nc.vector.tensor_tensor(
                            out=oh[:],
                            in0=bslot_t[:, c:c + 1].to_broadcast([128, 128]),
                            in1=iota_f[:, t * 128:(t + 1) * 128],
                            op=mybir.AluOpType.is_equal)
                        nc.tensor.matmul(out=poolp[t][:], lhsT=oh[:], rhs=h2b[:],
                                         start=(c == 0), stop=(c == COLS - 1))
                for t in (range(gtiles) if _stage >= 8 else []):
                    po = wpool.tile([128, HID], f32, tag="po")
                    nc.vector.tensor_copy(po[:], poolp[t][:])
                    nc.sync.dma_start(poolsum[t * 128:(t + 1) * 128, :], po[:])

    if _compile:
        nc.compile()
    return nc, nslots


def _get_program(K, gslot):
    key = (tuple(K), gslot)
    if key not in _prog_cache:
        _prog_cache[key] = _build_program(K, gslot)
    return _prog_cache[key]


def _warmup():
    """Compile the static program and run it once on zero inputs so the
    NEFF/executable caches are hot before the first real call."""
    nc, nslots = _get_program(K_STATIC, GSLOT_STATIC)
    zmaps = []
    for _ in range(NCORES):
        zmaps.append({
            "xpT": np.zeros((IN_CH, NLOC), ml_dtypes.bfloat16),
            "idx": np.zeros((128, nslots), np.int32),
            "dinv": np.zeros((128, COLS), np.float32),
            "bslot": np.zeros((128, COLS), np.float32),
            "w1": np.zeros((IN_CH, HID), ml_dtypes.bfloat16),
            "w2": np.zeros((HID, HID), ml_dtypes.bfloat16),
            "b1": np.zeros((HID, 1), np.float32),
            "b2rep": np.zeros((128, HID), np.float32),
        })
    from concourse.bass_utils import run_bass_kernel_spmd
    run_bass_kernel_spmd(nc, zmaps, core_ids=list(range(NCORES)))


def kernel(x, edge_index, batch, W1, b1, W2, b2, Wfc, bfc):
    from concourse.bass_utils import run_bass_kernel_spmd

    x = np.asarray(x, np.float32)
    src = np.asarray(edge_index[0]).astype(np.int64)
    dst = np.asarray(edge_index[1]).astype(np.int64)
    batch = np.asarray(batch).astype(np.int64)
    W1 = np.asarray(W1, np.float32); b1 = np.asarray(b1, np.float32)
    W2 = np.asarray(W2, np.float32); b2 = np.asarray(b2, np.float32)
    Wfc = np.asarray(Wfc, np.float32); bfc = np.asarray(bfc, np.float32)
    assert x.shape == (N_NODES, IN_CH) and src.shape == (N_EDGES,)

    # ---- host preprocessing ----
    deg = np.bincount(dst, minlength=N_NODES).astype(np.float32) + 1.0
    dinv = 1.0 / np.sqrt(deg)
    xp = (x * dinv[:, None])

    order = np.argsort(dst, kind="stable")
    dst_s = dst[order]
    src_s = src[order]
    starts = np.searchsorted(dst_s, np.arange(N_NODES + 1))

    ldeg_all = (starts[1:] - starts[:-1]).astype(np.int64)

    # per-shard degree ranking and required capacities
    rank_to_local = np.empty((NCORES, NLOC), np.int64)
    Kc_act = np.zeros((NCORES, COLS), np.int64)
    for d in range(NCORES):
        ld = ldeg_all[d * NLOC:(d + 1) * NLOC]
        r2l = np.argsort(-ld, kind="stable")
        rank_to_local[d] = r2l
        Kc_act[d] = ld[r2l].reshape(COLS, 128).max(axis=1)
    Kc_need = Kc_act.max(axis=0)

    # graph slots (dense per-shard numbering; works for any batch)
    gmaps = []
    maxg = 0
    for d in range(NCORES):
        b = batch[d * NLOC:(d + 1) * NLOC]
        gp = np.unique(b)
        gmaps.append(gp)
        maxg = max(maxg, len(gp))

    if np.all(Kc_need <= np.asarray(K_STATIC)) and maxg <= GSLOT_STATIC:
        K, gslot = K_STATIC, GSLOT_STATIC
    else:  # slow path: custom capacities
        K = tuple(int(k) + 2 for k in np.maximum(Kc_need, 4))
        gslot = ((maxg + 127) // 128) * 128
    nc, nslots = _get_program(K, gslot)
    _, slotpos = _slot_schedule(K)

    # global table row of each node: row = shard_base + (rank%128)*COLS + rank//128
    rowof = np.empty(N_NODES, np.int64)
    ranks = np.arange(NLOC)
    rowcode = (ranks % 128) * COLS + ranks // 128
    for d in range(NCORES):
        rowof[d * NLOC + rank_to_local[d]] = d * NLOC + rowcode
    src_rows = rowof[src_s].astype(np.int32)

    # per-edge (partition, slot) coordinates
    rank_of = np.empty((NCORES, NLOC), np.int64)
    for d in range(NCORES):
        rank_of[d, rank_to_local[d]] = ranks

    in_maps = []
    W1b = W1.astype(ml_dtypes.bfloat16)
    W2b = W2.astype(ml_dtypes.bfloat16)
    b1c = b1.reshape(HID, 1).astype(np.float32)
    b2r = np.broadcast_to(b2, (128, HID)).astype(np.float32).copy()
    for d in range(NCORES):
        lo, hi = starts[d * NLOC], starts[(d + 1) * NLOC]
        e_dst = dst_s[lo:hi] - d * NLOC            # local dst node
        e_j = np.arange(lo, hi) - starts[dst_s[lo:hi]]  # index within node
        e_rank = rank_of[d, e_dst]
        e_c = e_rank // 128
        e_p = e_rank % 128
        ia = np.full((128, nslots), PAD_ROW, np.int32)
        ia[e_p, slotpos[e_c, e_j]] = src_rows[lo:hi]

        perm = rank_to_local[d].reshape(COLS, 128).T   # [128, COLS] local ids
        xs = xp[d * NLOC:(d + 1) * NLOC]
        arr = xs[perm]                                  # [128, COLS, IN_CH]
        xpT_a = np.ascontiguousarray(
            arr.transpose(2, 1, 0).reshape(IN_CH, NLOC)).astype(ml_dtypes.bfloat16)
        dinv_a = dinv[d * NLOC:(d + 1) * NLOC][perm].astype(np.float32)
        bs = np.searchsorted(gmaps[d], batch[d * NLOC:(d + 1) * NLOC])
        bslot_a = bs[perm].astype(np.float32)

        in_maps.append({
            "xpT": xpT_a,
            "idx": ia,
            "dinv": dinv_a,
            "bslot": bslot_a,
            "w1": W1b,
            "w2": W2b,
            "b1": b1c,
            "b2rep": b2r,
        })

    # ---- launch ----
    t0 = time.perf_counter()
    res = run_bass_kernel_spmd(nc, in_maps, core_ids=list(range(NCORES)))
    LAST_EXEC_WALLS.append(time.perf_counter() - t0)

    # ---- host postprocess: combine pools, FC, sigmoid ----
    sums = np.zeros((N_GRAPHS, HID), np.float64)
    for d in range(NCORES):
        ps = res.results[d]["poolsum"]
        gp = gmaps[d]
        sums[gp] += ps[:len(gp)].astype(np.float64)
    cnt = np.bincount(batch, minlength=N_GRAPHS).astype(np.float64)
    g = sums / np.maximum(cnt, 1.0)[:, None]
    logits = g @ Wfc.astype(np.float64) + bfc.astype(np.float64)
    return (1.0 / (1.0 + np.exp(-logits))).astype(np.float32)


import os as _os
if _os.environ.get("KERNEL_NO_WARMUP") != "1":
    _warmup()


# revision 17
# speedup vs baseline: 18.8760x; 2.2523x over previous
import sys
sys.path.insert(0, "/opt/trn_rl_repo")
import time
import numpy as np
import ml_dtypes

# ---- problem constants (hardcoded; kernel.py must be self-contained) ----
N_NODES = 131072
N_EDGES = 2097152
N_GRAPHS = 2048
IN_CH, HID, OUT = 12, 64, 4
NCORES = 8
NLOC = N_NODES // NCORES      # 16384 nodes per core (fixed ranges)
COLS = NLOC // 128            # 128 columns of 128 nodes
PAD_ROW = N_NODES             # zero row appended to the gather tables

# Static per-column slot capacities (max in-degree of the column after
# degree-sorting node ranks, maxed over the 8 shards) for the reference
# input distribution, plus safety margin. If actual inputs exceed this,
# a custom program is built at runtime (slow path, still correct).
_K_SEED = [36, 27, 25, 25, 24, 24, 23, 23, 22, 22, 22, 22, 21, 21, 21, 21,
           21, 21, 20, 20, 20, 20, 20, 20, 20, 19, 19, 19, 19, 19, 19, 19,
           19, 19, 18, 18, 18, 18, 18, 18, 18, 18, 18, 18, 18, 17, 17, 17,
           17, 17, 17, 17, 17, 17, 17, 17, 16, 16, 16, 16, 16, 16, 16, 16,
           16, 16, 16, 16, 16, 15, 15, 15, 15, 15, 15, 15, 15, 15, 15, 15,
           15, 15, 15, 14, 14, 14, 14, 14, 14, 14, 14, 14, 14, 14, 13, 13,
           13, 13, 13, 13, 13, 13, 13, 13, 12, 12, 12, 12, 12, 12, 12, 12,
           12, 11, 11, 11, 11, 11, 11, 10, 10, 10, 10, 9, 9, 9, 8, 7]
K_STATIC = tuple(int(np.ceil(k * 1.18)) + 2 for k in _K_SEED)
GSLOT_STATIC = 384            # graph slots per core (3 PSUM tiles of 128)

LAST_EXEC_WALLS = []
_prog_cache = {}


def _slot_schedule(K):
    """j-major slot emission order + per-(col, j) position table."""
    slots = []
    slotpos = np.full((COLS, max(K)), -1, np.int64)
    for j in range(max(K)):
        for c in range(COLS):
            if K[c] > j:
                slotpos[c, j] = len(slots)
                slots.append((c, j))
    return slots, slotpos


def _build_program(K, gslot, _stage=99, _compile=True):
    import concourse.bass as bass
    import concourse.bacc as bacc
    import concourse.tile as tile
    import concourse.mybir as mybir
    from concourse.masks import make_identity

    slots, _ = _slot_schedule(K)
    nslots = len(slots)
    gtiles = gslot // 128
    assert gslot % 128 == 0

    nc = bacc.Bacc("TRN2", target_bir_lowering=False, debug=False,
                   num_devices=NCORES)
    f32, bf16, i32 = mybir.dt.float32, mybir.dt.bfloat16, mybir.dt.int32

    xpT = nc.dram_tensor("xpT", [IN_CH, NLOC], bf16, kind="ExternalInput").ap()
    idx = nc.dram_tensor("idx", [128, nslots], i32, kind="ExternalInput").ap()
    dinv = nc.dram_tensor("dinv", [128, COLS], f32, kind="ExternalInput").ap()
    bslot = nc.dram_tensor("bslot", [128, COLS], f32, kind="ExternalInput").ap()
    w1 = nc.dram_tensor("w1", [IN_CH, HID], bf16, kind="ExternalInput").ap()
    w2 = nc.dram_tensor("w2", [HID, HID], bf16, kind="ExternalInput").ap()
    b1 = nc.dram_tensor("b1", [HID, 1], f32, kind="ExternalInput").ap()
    b2rep = nc.dram_tensor("b2rep", [128, HID], f32, kind="ExternalInput").ap()

    t1loc = nc.dram_tensor("t1loc", [NLOC, HID], bf16, kind="Internal").ap()
    t2loc = nc.dram_tensor("t2loc", [NLOC, HID], bf16, kind="Internal").ap()
    t1full = nc.dram_tensor("t1full", [N_NODES + 1, HID], bf16,
                            kind="Internal", addr_space="Shared").ap()
    t2full = nc.dram_tensor("t2full", [N_NODES + 1, HID], bf16,
                            kind="Internal", addr_space="Shared").ap()
    poolsum = nc.dram_tensor("poolsum", [gslot, HID], f32,
                             kind="ExternalOutput").ap()

    RG = [list(range(NCORES))]
    relu = mybir.ActivationFunctionType.Relu

    with tile.TileContext(nc) as tc:
        with tc.tile_pool(name="const", bufs=1) as cpool, \
             tc.tile_pool(name="work", bufs=3) as wpool:
            # ---- constant loads ----
            idx_t = cpool.tile([128, nslots], i32, name="idx_t")
            nc.sync.dma_start(idx_t[:], idx[:])
            xpT_t = cpool.tile([IN_CH, NLOC], bf16, name="xpT_t")
            nc.sync.dma_start(xpT_t[:], xpT[:])
            dinv_t = cpool.tile([128, COLS], f32, name="dinv_t")
            nc.sync.dma_start(dinv_t[:], dinv[:])
            bslot_t = cpool.tile([128, COLS], f32, name="bslot_t")
            nc.sync.dma_start(bslot_t[:], bslot[:])
            w1_t = cpool.tile([IN_CH, HID], bf16, name="w1_t")
            nc.sync.dma_start(w1_t[:], w1[:])
            w2_t = cpool.tile([HID, HID], bf16, name="w2_t")
            nc.sync.dma_start(w2_t[:], w2[:])
            b1_t = cpool.tile([HID, 1], f32, name="b1_t")
            nc.sync.dma_start(b1_t[:], b1[:])
            b2rep_t = cpool.tile([128, HID], f32, name="b2rep_t")
            nc.sync.dma_start(b2rep_t[:], b2rep[:])

            ident = cpool.tile([128, 128], f32, name="ident")
            make_identity(nc, ident[:])
            iota_i = cpool.tile([128, gslot], i32, name="iota_i")
            nc.gpsimd.iota(iota_i[:], pattern=[[1, gslot]], base=0,
                           channel_multiplier=0)
            iota_f = cpool.tile([128, gslot], f32, name="iota_f")
            nc.vector.tensor_copy(iota_f[:], iota_i[:])

            zrow = cpool.tile([1, HID], bf16, name="zrow")
            nc.vector.memset(zrow[:], 0.0)
            nc.sync.dma_start(t1full[PAD_ROW:PAD_ROW + 1, :], zrow[:])
            nc.sync.dma_start(t2full[PAD_ROW:PAD_ROW + 1, :], zrow[:])

            ylocal = cpool.tile([128, COLS * HID], bf16, name="ylocal")
            t2local = cpool.tile([128, COLS * HID], bf16, name="t2local")
            A = [cpool.tile([128, HID], f32, name=f"A{c}") for c in range(COLS)]
            B = [cpool.tile([128, HID], f32, name=f"B{c}") for c in range(COLS)]
            for c in range(COLS):
                nc.vector.memset(A[c][:], 0.0)
                nc.vector.memset(B[c][:], 0.0)

            # ---- stage A: y = xp @ W1 per column (all in transposed form) ----
            with tc.tile_pool(name="psA", bufs=4, space="PSUM") as psA:
                if _stage < 2:
                    slots2, cols2 = [], 0
                else:
                    slots2, cols2 = slots, COLS
                for c in range(cols2):
                    yp = psA.tile([128, HID], f32, space="PSUM", tag="yp")
                    nc.tensor.matmul(out=yp[:], lhsT=xpT_t[:, c * 128:(c + 1) * 128],
                                     rhs=w1_t[:], start=True, stop=True)
                    nc.scalar.activation(ylocal[:, c * HID:(c + 1) * HID], yp[:],
                                         mybir.ActivationFunctionType.Copy)
            # write T1 local chunk (row = p*COLS + c) and all-gather
            t1loc_v = t1loc[:, :].rearrange("(p c) f -> p (c f)", p=128)
            if _stage >= 2:
                nc.sync.dma_start(t1loc_v, ylocal[:])
            if _stage >= 3:
                nc.gpsimd.collective_compute(
                    kind="AllGather", op=mybir.AluOpType.bypass, replica_groups=RG,
                    ins=[t1loc[:, :]], outs=[t1full[0:N_NODES, :]])

            # ---- stage C: layer-1 gather-accumulate ----
            for pos, (c, j) in (enumerate(slots) if _stage >= 4 else []):
                nc.gpsimd.indirect_dma_start(
                    out=A[c][:, :], out_offset=None, in_=t1full[:],
                    in_offset=bass.IndirectOffsetOnAxis(
                        ap=idx_t[:, pos:pos + 1], axis=0),
                    compute_op=mybir.AluOpType.add)

            with tc.tile_pool(name="psC", bufs=4, space="PSUM") as psC:
                for c in (range(COLS) if _stage >= 5 else []):
                    cs = slice(c * HID, (c + 1) * HID)
                    nc.vector.tensor_add(A[c][:], A[c][:], ylocal[:, cs])
                    nc.scalar.mul(A[c][:], A[c][:], dinv_t[:, c:c + 1])
                    tp = psC.tile([HID, 128], f32, space="PSUM", tag="tp")
                    nc.tensor.transpose(tp[:], A[c][:], ident[:])
                    h1T = wpool.tile([HID, 128], bf16, tag="h1T")
                    nc.scalar.activation(h1T[:], tp[:], relu, bias=b1_t[:, 0:1])
                    t2p = psC.tile([128, HID], f32, space="PSUM", tag="t2p")
                    nc.tensor.matmul(out=t2p[:], lhsT=h1T[:], rhs=w2_t[:],
                                     start=True, stop=True)
                    nc.scalar.mul(t2local[:, cs], t2p[:], dinv_t[:, c:c + 1])

            t2loc_v = t2loc[:, :].rearrange("(p c) f -> p (c f)", p=128)
            if _stage >= 6:
                nc.sync.dma_start(t2loc_v, t2local[:])
                nc.gpsimd.collective_compute(
                    kind="AllGather", op=mybir.AluOpType.bypass, replica_groups=RG,
                    ins=[t2loc[:, :]], outs=[t2full[0:N_NODES, :]])

            # ---- stage E: layer-2 gather-accumulate ----
            for pos, (c, j) in (enumerate(slots) if _stage >= 7 else []):
                nc.gpsimd.indirect_dma_start(
                    out=B[c][:, :], out_offset=None, in_=t2full[:],
                    in_offset=bass.IndirectOffsetOnAxis(
                        ap=idx_t[:, pos:pos + 1], axis=0),
                    compute_op=mybir.AluOpType.add)

            with tc.tile_pool(name="psE", bufs=1, space="PSUM") as psE:
                poolp = [psE.tile([128, HID], f32, space="PSUM", name=f"pool{t}")
                         for t in range(gtiles)]
                for c in (range(COLS) if _stage >= 8 else []):
                    cs = slice(c * HID, (c + 1) * HID)
                    nc.vector.tensor_add(B[c][:], B[c][:], t2local[:, cs])
                    nc.scalar.mul(B[c][:], B[c][:], dinv_t[:, c:c + 1])
                    h2 = wpool.tile([128, HID], f32, tag="h2")
                    nc.vector.tensor_add(h2[:], B[c][:], b2rep_t[:])
                    h2b = wpool.tile([128, HID], bf16, tag="h2b")
                    nc.vector.tensor_scalar_max(h2b[:], h2[:], 0.0)
                    for t in range(gtiles):
                        oh = wpool.tile([128, 128], bf16, tag=f"oh{t}")
                        nc.vector.tensor_tensor(
                            out=oh[:],
                            in0=bslot_t[:, c:c + 1].to_broadcast([128, 128]),
                            in1=iota_f[:, t * 128:(t + 1) * 128],
                            op=mybir.AluOpType.is_equal)
                        nc.tensor.matmul(out=poolp[t][:], lhsT=oh[:], rhs=h2b[:],
                                         start=(c == 0), stop=(c == COLS - 1))
                for t in (range(gtiles) if _stage >= 8 else []):
                    po = wpool.tile([128, HID], f32, tag="po")
                    nc.vector.tensor_copy(po[:], poolp[t][:])
                    nc.sync.dma_start(poolsum[t * 128:(t + 1) * 128, :], po[:])

    if _compile:
        nc.compile()
    return nc, nslots


def _get_program(K, gslot):
    key = (tuple(K), gslot)
    if key not in _prog_cache:
        _prog_cache[key] = _build_program(K, gslot)
    return _prog_cache[key]


_runner_cache = {}


def _make_runner(nc):
    """Like bass2jax.run_bass_via_pjrt, but the jitted executable is built
    once and cached, so repeat calls skip jax retrace + XLA re-compile."""
    if id(nc) in _runner_cache:
        return _runner_cache[id(nc)]
    import jax
    from jax.experimental.shard_map import shard_map
    from jax.sharding import Mesh, PartitionSpec
    from concourse import bass2jax, mybir

    bass2jax.install_neuronx_cc_hook()
    assert nc.dbg_addr is None
    partition_name = (nc.partition_id_tensor.name
                      if nc.partition_id_tensor else None)

    in_names, out_names, out_avals = [], [], []
    for alloc in nc.m.functions[0].allocations:
        if not isinstance(alloc, mybir.MemoryLocationSet):
            continue
        name = alloc.memorylocations[0].name
        if alloc.kind == "ExternalInput":
            if name != partition_name:
                in_names.append(name)
        elif alloc.kind == "ExternalOutput":
            shape = tuple(alloc.tensor_shape)
            dtype = mybir.dt.np(alloc.dtype)
            out_names.append(name)
            out_avals.append(jax.core.ShapedArray(shape, dtype))
    n_params = len(in_names)
    n_outs = len(out_avals)
    all_names = list(in_names) + list(out_names)
    if partition_name is not None:
        all_names.append(partition_name)
    donate = tuple(range(n_params, n_params + n_outs))

    def _body(*args):
        operands = list(args)
        if partition_name is not None:
            operands.append(bass2jax.partition_id_tensor())
        outs = bass2jax._bass_exec_p.bind(
            *operands,
            out_avals=tuple(out_avals),
            in_names=tuple(all_names),
            out_names=tuple(out_names),
            lowering_input_output_aliases=(),
            sim_require_finite=True,
            sim_require_nnan=True,
            nc=nc,
        )
        return tuple(outs)

    devices = jax.devices()[:NCORES]
    mesh = Mesh(np.asarray(devices), ("core",))
    in_specs = (PartitionSpec("core"),) * (n_params + n_outs)
    out_specs = (PartitionSpec("core"),) * n_outs
    sharded = jax.jit(
        shard_map(_body, mesh=mesh, in_specs=in_specs, out_specs=out_specs,
                  check_rep=False),
        donate_argnums=donate, keep_unused=True)

    def run(in_maps):
        concat_in = [
            np.concatenate([np.asarray(m[name]) for m in in_maps], axis=0)
            for name in in_names
        ]
        concat_zeros = [
            np.zeros((NCORES * a.shape[0], *a.shape[1:]), a.dtype)
            for a in out_avals
        ]
        out_arrs = sharded(*concat_in, *concat_zeros)
        return [
            {name: np.asarray(out_arrs[i]).reshape(NCORES, *out_avals[i].shape)[c]
             for i, name in enumerate(out_names)}
            for c in range(NCORES)
        ]

    _runner_cache[id(nc)] = run
    return run


def _warmup():
    """Compile the static program and run it once on zero inputs so the
    NEFF/executable caches are hot before the first real call."""
    nc, nslots = _get_program(K_STATIC, GSLOT_STATIC)
    zmaps = []
    for _ in range(NCORES):
        zmaps.append({
            "xpT": np.zeros((IN_CH, NLOC), ml_dtypes.bfloat16),
            "idx": np.zeros((128, nslots), np.int32),
            "dinv": np.zeros((128, COLS), np.float32),
            "bslot": np.zeros((128, COLS), np.float32),
            "w1": np.zeros((IN_CH, HID), ml_dtypes.bfloat16),
            "w2": np.zeros((HID, HID), ml_dtypes.bfloat16),
            "b1": np.zeros((HID, 1), np.float32),
            "b2rep": np.zeros((128, HID), np.float32),
        })
    _make_runner(nc)(zmaps)


def kernel(x, edge_index, batch, W1, b1, W2, b2, Wfc, bfc):
    x = np.asarray(x, np.float32)
    src = np.asarray(edge_index[0]).astype(np.int64)
    dst = np.asarray(edge_index[1]).astype(np.int64)
    batch = np.asarray(batch).astype(np.int64)
    W1 = np.asarray(W1, np.float32); b1 = np.asarray(b1, np.float32)
    W2 = np.asarray(W2, np.float32); b2 = np.asarray(b2, np.float32)
    Wfc = np.asarray(Wfc, np.float32); bfc = np.asarray(bfc, np.float32)
    assert x.shape == (N_NODES, IN_CH) and src.shape == (N_EDGES,)

    # ---- host preprocessing ----
    deg = np.bincount(dst, minlength=N_NODES).astype(np.float32) + 1.0
    dinv = 1.0 / np.sqrt(deg)
    xp = (x * dinv[:, None])

    # stable dst-grouping via packed radix sort (order within a node is
    # irrelevant); src < 2^17 fits in the low bits
    key = np.sort((dst << 17) | src, kind="stable")
    dst_s = key >> 17
    src_s = key & ((1 << 17) - 1)
    starts = np.searchsorted(dst_s, np.arange(N_NODES + 1))

    ldeg_all = (starts[1:] - starts[:-1]).astype(np.int64)

    # per-shard degree ranking and required capacities
    rank_to_local = np.empty((NCORES, NLOC), np.int64)
    Kc_act = np.zeros((NCORES, COLS), np.int64)
    for d in range(NCORES):
        ld = ldeg_all[d * NLOC:(d + 1) * NLOC]
        r2l = np.argsort(-ld, kind="stable")
        rank_to_local[d] = r2l
        Kc_act[d] = ld[r2l].reshape(COLS, 128).max(axis=1)
    Kc_need = Kc_act.max(axis=0)

    # graph slots (dense per-shard numbering; works for any batch)
    gmaps = []
    maxg = 0
    for d in range(NCORES):
        b = batch[d * NLOC:(d + 1) * NLOC]
        gp = np.unique(b)
        gmaps.append(gp)
        maxg = max(maxg, len(gp))

    if np.all(Kc_need <= np.asarray(K_STATIC)) and maxg <= GSLOT_STATIC:
        K, gslot = K_STATIC, GSLOT_STATIC
    else:  # slow path: custom capacities
        K = tuple(int(k) + 2 for k in np.maximum(Kc_need, 4))
        gslot = ((maxg + 127) // 128) * 128
    nc, nslots = _get_program(K, gslot)
    _, slotpos = _slot_schedule(K)

    # global table row of each node: row = shard_base + (rank%128)*COLS + rank//128
    rowof = np.empty(N_NODES, np.int64)
    ranks = np.arange(NLOC)
    rowcode = (ranks % 128) * COLS + ranks // 128
    for d in range(NCORES):
        rowof[d * NLOC + rank_to_local[d]] = d * NLOC + rowcode
    src_rows = rowof[src_s].astype(np.int32)

    # per-edge (partition, slot) coordinates
    rank_of = np.empty((NCORES, NLOC), np.int64)
    for d in range(NCORES):
        rank_of[d, rank_to_local[d]] = ranks

    in_maps = []
    W1b = W1.astype(ml_dtypes.bfloat16)
    W2b = W2.astype(ml_dtypes.bfloat16)
    b1c = b1.reshape(HID, 1).astype(np.float32)
    b2r = np.broadcast_to(b2, (128, HID)).astype(np.float32).copy()
    for d in range(NCORES):
        lo, hi = starts[d * NLOC], starts[(d + 1) * NLOC]
        e_dst = dst_s[lo:hi] - d * NLOC            # local dst node
        e_j = np.arange(lo, hi) - starts[dst_s[lo:hi]]  # index within node
        e_rank = rank_of[d, e_dst]
        e_c = e_rank // 128
        e_p = e_rank % 128
        ia = np.full((128, nslots), PAD_ROW, np.int32)
        ia[e_p, slotpos[e_c, e_j]] = src_rows[lo:hi]

        perm = rank_to_local[d].reshape(COLS, 128).T   # [128, COLS] local ids
        xs = xp[d * NLOC:(d + 1) * NLOC]
        arr = xs[perm]                                  # [128, COLS, IN_CH]
        xpT_a = np.ascontiguousarray(
            arr.transpose(2, 1, 0).reshape(IN_CH, NLOC)).astype(ml_dtypes.bfloat16)
        dinv_a = dinv[d * NLOC:(d + 1) * NLOC][perm].astype(np.float32)
        bs = np.searchsorted(gmaps[d], batch[d * NLOC:(d + 1) * NLOC])
        bslot_a = bs[perm].astype(np.float32)

        in_maps.append({
            "xpT": xpT_a,
            "idx": ia,
            "dinv": dinv_a,
            "bslot": bslot_a,
            "w1": W1b,
            "w2": W2b,
            "b1": b1c,
            "b2rep": b2r,
        })

    # ---- launch ----
    run = _make_runner(nc)
    t0 = time.perf_counter()
    results = run(in_maps)
    LAST_EXEC_WALLS.append(time.perf_counter() - t0)

    # ---- host postprocess: combine pools, FC, sigmoid ----
    sums = np.zeros((N_GRAPHS, HID), np.float64)
    for d in range(NCORES):
        ps = results[d]["poolsum"]
        gp = gmaps[d]
        sums[gp] += ps[:len(gp)].astype(np.float64)
    cnt = np.bincount(batch, minlength=N_GRAPHS).astype(np.float64)
    g = sums / np.maximum(cnt, 1.0)[:, None]
    logits = g @ Wfc.astype(np.float64) + bfc.astype(np.float64)
    return (1.0 / (1.0 + np.exp(-logits))).astype(np.float32)


import os as _os
if _os.environ.get("KERNEL_NO_WARMUP") != "1":
    _warmup()
